# revision 28
# baseline (speedup 1.0000x reference)
# Bass/Tile TRN2 kernel for nn_Attn_2130303779132 (general-score attention).
#
# Math: reference computes
#   proj = einsum('sbh,kh->sbk', enc, W) + b        # (S,B,H) huge matmul
#   energies[b,s] = <hidden[b], proj[s,b]>          # (B,S)
#   out = softmax(energies, axis=-1)
# Algebraically:
#   energies[b,s] = sum_h enc[s,b,h] * v[b,h] + (hidden[b]·bias)
# with v = hidden @ W.  The bias term is constant across s, so softmax
# removes it exactly.
#
# v4 design (fp16 stream, DVE + PE split):
#   * enc is uploaded fp16 (halves HBM traffic; ~16 MiB + 2 MiB W per
#     core is the DMA floor).  The s-range is split per batch:
#     - columns 0..15 (s < 2048): natural layout [s-part, h-free];
#       each E column is one fused DVE scalar_tensor_tensor job
#       (multiply by the broadcast v, accumulate over h).  ~1.13 us/col.
#     - columns 16..31 (s >= 2048): host-transposed layout
#       [h-part, s-free]; each E column is a chain of 8 PE matmuls
#       (lhsT = enc chunk, rhs = v column) accumulating in PSUM.
#       ~1.17 us/col, dominated by instruction dispatch.
#     The two engines run concurrently, each well under the DMA stream.
#   * Tiles are interleaved nat/tr and tapered so the post-DMA tail is
#     only a couple of jobs per engine.
#   * Softmax: partition reductions via PE transpose + ones-matmul;
#     1/Z folded into the transposed output copy.  No gpsimd anywhere.
#
# Sharding: data-parallel over batch. 8 cores x 2 batches each.
# W replicated; no collectives.

import numpy as np

import concourse.bacc as bacc
import concourse.bass as bass
import concourse.tile as tile
from concourse import mybir
from concourse.bass_utils import run_bass_kernel_spmd

S, B, H = 4096, 16, 1024
NCORES = 8
BL = B // NCORES          # local batches per core = 2
P = 128                   # partitions
KC = H // P               # 8 h-chunks
NCHUNK = S // P           # 32 s-chunks of 128
ND = 16                   # DVE columns per batch (natural layout)
NP_ = NCHUNK - ND         # PE columns per batch (transposed layout)
SN = ND * P               # s-extent of the natural half (2048)
# Per-batch DMA tile schedule: ("nat", col0, ncols) | ("tr", k0, nk).
# Interleaved and tapered so both engines stay fed and only the last
# couple of jobs depend on the final transfer.
TILES = [
    ("nat", 0, 10),
    ("nat", 10, 4),
    ("tr", 0, 3),
    ("tr", 3, 3),
    ("nat", 14, 2),
    ("tr", 6, 2),
]
F32 = mybir.dt.float32
F16 = mybir.dt.float16


def build_bass(loop_n: int = 1) -> bass.Bass:
    """loop_n > 1 wraps the whole kernel body in an on-device For loop —
    used only for steady-state timing (amortizes RPC/launch overhead)."""
    nc = bacc.Bacc("TRN2", target_bir_lowering=False, debug=False,
                   num_devices=NCORES)

    enc_nat = nc.dram_tensor("enc_nat", (BL, SN, H), F16,
                             kind="ExternalInput").ap()
    enc_tr = nc.dram_tensor("enc_tr", (BL, KC, P, S - SN), F16,
                            kind="ExternalInput").ap()
    hid = nc.dram_tensor("hid", (BL, H), F16, kind="ExternalInput").ap()
    w = nc.dram_tensor("w", (H, H), F16, kind="ExternalInput").ap()
    selc = nc.dram_tensor("selc", (BL, BL * P), F16,
                          kind="ExternalInput").ap()
    eye = nc.dram_tensor("eye", (P, P), F32, kind="ExternalInput").ap()
    out = nc.dram_tensor("out", (BL, S), F32, kind="ExternalOutput").ap()

    with tile.TileContext(nc) as tc:
        with (
            tc.tile_pool(name="consts", bufs=1) as consts,
            tc.tile_pool(name="wpool", bufs=1) as wpool,
            tc.tile_pool(name="encpool", bufs=2) as encpool,
            tc.tile_pool(name="scratch", bufs=2) as scratch,
            tc.tile_pool(name="small", bufs=2) as small,
            tc.tile_pool(name="psumc", bufs=1, space="PSUM") as psumc,
            tc.tile_pool(name="psume", bufs=1, space="PSUM") as psume,
            tc.tile_pool(name="psumt", bufs=1, space="PSUM") as psumt,
        ):
            pools = (consts, wpool, encpool, scratch, small,
                     psumc, psume, psumt)

            def body():
                build_body(nc, pools, enc_nat, enc_tr, hid, w, selc, eye,
                           out)

            if loop_n == 1:
                body()
            else:
                with tc.For_i(0, loop_n, 1):
                    body()

    nc.compile()
    return nc


def build_body(nc, pools, enc_nat, enc_tr, hid, w, selc, eye, out):
    consts, wpool, encpool, scratch, small, psumc, psume, psumt = pools

    # ---------------- prologue: v = hidden @ W, both layouts ----------
    # The prologue loads go on the SAME sync ring as the enc stream,
    # ahead of it: ring FIFO order guarantees the v-chain inputs (which
    # gate every DVE job) land before the first 2.5 MB enc tile.  Small
    # tensors first so the hidden transposes start immediately.
    ident = consts.tile([P, P], F32, tag="ident")
    nc.sync.dma_start(out=ident, in_=eye)
    hid_sb = consts.tile([BL, H], F16, tag="hid")
    nc.sync.dma_start(out=hid_sb, in_=hid)
    selc_sb = consts.tile([BL, BL * P], F16, tag="selc")
    nc.sync.dma_start(out=selc_sb, in_=selc)
    w_tiles = []
    for i in range(KC):
        w_t = wpool.tile([P, H], F16, tag=f"w{i}", name=f"w{i}")
        nc.sync.dma_start(out=w_t, in_=w[i * P: (i + 1) * P, :])
        w_tiles.append(w_t)

    ident16 = consts.tile([P, P], F16, tag="ident16")
    nc.scalar.copy(out=ident16, in_=ident)
    ones_row = consts.tile([1, P], F32, tag="ones_row")
    nc.vector.memset(ones_row, 1.0)
    ones_col = consts.tile([P, 1], F32, tag="ones_col")
    nc.vector.memset(ones_col, 1.0)
    # Preload the Exp table while the prologue runs so the first real
    # softmax doesn't eat the LoadActFuncSet latency.
    actwarm = consts.tile([1, 1], F32, tag="actwarm")
    nc.scalar.activation(out=actwarm, in_=ones_row[:, 0:1],
                         func=mybir.ActivationFunctionType.Exp)

    # hidden^T via PE transposes: hT[:, 2i + b] = hidden[b, i*128 : ...]
    psum_hT = psumc.tile([P, BL * KC], F16, tag="hT")
    for i in range(KC):
        nc.tensor.transpose(
            out=psum_hT[:, BL * i: BL * i + BL],
            in_=hid_sb[:, i * P: (i + 1) * P],
            identity=ident16[0:BL, 0:BL],
        )
    hT_sb = consts.tile([P, BL * KC], F16, tag="hTsb")
    nc.scalar.copy(out=hT_sb, in_=psum_hT)

    # v = hidden @ W as [BL, H] (v[b,h] = sum_k hid[b,k] W[k,h])
    psum_v = psumc.tile([BL, H], F32, tag="v")
    for j in range(H // 512):
        for i in range(KC):
            nc.tensor.matmul(
                out=psum_v[:, j * 512: (j + 1) * 512],
                lhsT=hT_sb[:, BL * i: BL * i + BL],
                rhs=w_tiles[i][:, j * 512: (j + 1) * 512],
                start=(i == 0),
                stop=(i == KC - 1),
            )
    v_sb16 = consts.tile([BL, H], F16, tag="v16")
    nc.scalar.copy(out=v_sb16, in_=psum_v)

    # vT[:, BL*i + b] = v[b, i*128:(i+1)*128] for the PE path
    psum_vT = psumc.tile([P, BL * KC], F16, tag="vT")
    for i in range(KC):
        nc.tensor.transpose(
            out=psum_vT[:, BL * i: BL * i + BL],
            in_=v_sb16[:, i * P: (i + 1) * P],
            identity=ident16[0:BL, 0:BL],
        )
    vT16 = consts.tile([P, BL * KC], F16, tag="vT16")
    nc.scalar.copy(out=vT16, in_=psum_vT)

    # vb[:, b*H:(b+1)*H] = v[b, :] replicated to all 128 partitions via
    # a selector matmul (selc row b is all-ones over b's 128 columns).
    vb16 = consts.tile([P, BL * H], F16, tag="vb16")
    psum_vb = psumc.tile([P, 512], F32, tag="vb")
    for b in range(BL):
        for j in range(H // 512):
            nc.tensor.matmul(
                out=psum_vb,
                lhsT=selc_sb[:, b * P: (b + 1) * P],
                rhs=v_sb16[:, j * 512: (j + 1) * 512],
                start=True,
                stop=True,
            )
            nc.scalar.copy(
                out=vb16[:, b * H + j * 512: b * H + (j + 1) * 512],
                in_=psum_vb,
            )

    # ---------------- main loop: E columns on DVE and PE --------------
    # E[b][p, c] = sum_h enc[c*128+p, b, h] * v[b, h]
    E = [
        consts.tile([P, NCHUNK], F32, tag=f"E{b}", name=f"E{b}")
        for b in range(BL)
    ]
    # PE half accumulates here; column b*NP_+sc is batch b's col 16+sc.
    psum_E = psume.tile([P, BL * NP_], F32, tag="Epe")
    npairs = NP_ * KC  # PE matmuls per batch
    for b in range(BL):
        pair = 0
        for kind, a0, n in TILES:
            if kind == "nat":
                et = encpool.tile([P, n, H], F16, tag=f"nat{n}")
                nc.sync.dma_start(
                    out=et,
                    in_=enc_nat[b, a0 * P: (a0 + n) * P, :].rearrange(
                        "(q p) h -> p q h", p=P
                    ),
                )
                for q in range(n):
                    c = a0 + q
                    prod = scratch.tile([P, H], F16, tag="prod")
                    nc.vector.scalar_tensor_tensor(
                        out=prod, in0=et[:, q, :], scalar=1.0,
                        in1=vb16[:, b * H: (b + 1) * H],
                        op0=mybir.AluOpType.mult,
                        op1=mybir.AluOpType.mult,
                        accum_out=E[b][:, c: c + 1],
                    )
            else:
                et = encpool.tile([P, n, S - SN], F16, tag=f"tr{n}")
                nc.sync.dma_start(
                    out=et,
                    in_=enc_tr[b, a0: a0 + n].rearrange("k p s -> p k s"),
                )
                for sc in range(NP_):
                    for ki in range(n):
                        k = a0 + ki
                        nc.tensor.matmul(
                            out=psum_E[:, b * NP_ + sc: b * NP_ + sc + 1],
                            lhsT=et[:, ki, sc * P: (sc + 1) * P],
                            rhs=vT16[:, BL * k + b: BL * k + b + 1],
                            start=(pair == 0),
                            stop=(pair == npairs - 1),
                        )
                        pair += 1
        # merge the PE half into E and run the softmax for this batch
        nc.scalar.copy(
            out=E[b][:, ND:NCHUNK],
            in_=psum_E[:, b * NP_: (b + 1) * NP_],
        )
        softmax_store(nc, pools, E[b], out, b, ident, ones_row, ones_col)


def softmax_store(nc, pools, Eb, out, b, ident, ones_row, ones_col):
    consts, wpool, encpool, scratch, small, psumc, psume, psumt = pools

    # one PSUM bank carved into disjoint slices for all softmax
    # intermediates (lifetimes don't overlap where regions do).
    smx = psumt.tile([P, 136], F32, tag="smx")
    psum_rt = smx[0:1, 0:P]          # row-max transposed; dead before eT
    psum_eT = smx[0:NCHUNK, 0:P]     # exp(E-m)^T, written after rt dies
    psum_tot = smx[0:1, P: P + 1]
    psum_rtot = smx[0:NCHUNK, P + 1: P + 2]
    psum_negm = smx[0:P, P + 2: P + 3]

    # global max over S: free-dim max on DVE, partition max via PE
    # transpose + free-dim max, then broadcast back to [128, 1].
    rmax = small.tile([P, 1], F32, tag="rmax")
    nc.vector.tensor_reduce(
        out=rmax, in_=Eb, axis=mybir.AxisListType.X,
        op=mybir.AluOpType.max,
    )
    nc.tensor.transpose(out=psum_rt, in_=rmax, identity=ident)
    negm1 = small.tile([1, 1], F32, tag="negm1")
    nc.vector.tensor_reduce(
        out=negm1, in_=psum_rt, axis=mybir.AxisListType.X,
        op=mybir.AluOpType.max, negate=True,
    )
    nc.tensor.matmul(out=psum_negm, lhsT=ones_row, rhs=negm1,
                     start=True, stop=True)
    negm = small.tile([P, 1], F32, tag="negm")
    nc.scalar.copy(out=negm, in_=psum_negm)

    eexp = small.tile([P, NCHUNK], F32, tag="eexp")
    nc.scalar.activation(
        out=eexp, in_=Eb,
        func=mybir.ActivationFunctionType.Exp,
        bias=negm, scale=1.0,
    )
    # transpose the unnormalized numerator early; the 1/Z scale lands in
    # the final ACT copy as a per-partition scalar.
    nc.tensor.transpose(out=psum_eT, in_=eexp, identity=ident)

    # total = sum over S: free-dim sum on DVE, partition sum via ones-
    # matmul (contracts the partition axis), then reciprocal.
    rowsum = small.tile([P, 1], F32, tag="rowsum")
    nc.vector.tensor_reduce(
        out=rowsum, in_=eexp, axis=mybir.AxisListType.X,
        op=mybir.AluOpType.add,
    )
    nc.tensor.matmul(out=psum_tot, lhsT=rowsum, rhs=ones_col,
                     start=True, stop=True)
    rtot1 = small.tile([1, 1], F32, tag="rtot1")
    nc.vector.reciprocal(out=rtot1, in_=psum_tot)
    nc.tensor.matmul(out=psum_rtot, lhsT=ones_row[:, 0:NCHUNK], rhs=rtot1,
                     start=True, stop=True)
    rtot32 = small.tile([NCHUNK, 1], F32, tag="rtot32")
    nc.scalar.copy(out=rtot32, in_=psum_rtot)

    # out[b][sc*128 + p] = eexp[p, sc] / Z: scaled copy of the transpose
    pT_sb = small.tile([NCHUNK, P], F32, tag="pTsb")
    nc.scalar.activation(
        out=pT_sb, in_=psum_eT,
        func=mybir.ActivationFunctionType.Copy,
        scale=rtot32,
    )
    nc.scalar.dma_start(
        out=out[b].rearrange("(c p) -> c p", p=P), in_=pT_sb
    )


_NC_CACHE = None


def _get_nc() -> bass.Bass:
    global _NC_CACHE
    if _NC_CACHE is None:
        _NC_CACHE = build_bass()
    return _NC_CACHE


def make_in_maps(hidden, encoder_outputs, W):
    hidden = np.asarray(hidden, dtype=np.float32)
    encoder_outputs = np.asarray(encoder_outputs, dtype=np.float32)
    W = np.asarray(W, dtype=np.float32)
    # natural half (s < SN): (SN, B, H) -> (B, SN, H) fp16
    enc_nat = np.ascontiguousarray(
        encoder_outputs[:SN].transpose(1, 0, 2)
    ).astype(np.float16)
    # transposed half (s >= SN): (S-SN, B, H) -> (B, H, S-SN) fp16
    enc_tr = np.ascontiguousarray(
        encoder_outputs[SN:].transpose(1, 2, 0)
    ).astype(np.float16)
    w16 = np.ascontiguousarray(W).astype(np.float16)
    hid16 = hidden[0].astype(np.float16)
    eye = np.eye(P, dtype=np.float32)
    selc = np.zeros((BL, BL * P), dtype=np.float16)
    for b in range(BL):
        selc[b, b * P: (b + 1) * P] = 1.0
    in_maps = []
    for c in range(NCORES):
        in_maps.append(
            {
                "enc_nat": enc_nat[c * BL: (c + 1) * BL],
                "enc_tr": enc_tr[c * BL: (c + 1) * BL].reshape(
                    BL, KC, P, S - SN
                ),
                "hid": np.ascontiguousarray(hid16[c * BL: (c + 1) * BL]),
                "w": w16,
                "selc": selc,
                "eye": eye,
            }
        )
    return in_maps


def kernel(hidden, encoder_outputs, W, b, **run_kwargs):
    # `b` (the nn.Linear bias) shifts every energy row by a per-batch
    # constant, which softmax cancels exactly — unused on device.
    nc = _get_nc()
    in_maps = make_in_maps(hidden, encoder_outputs, W)
    res = run_bass_kernel_spmd(
        nc, in_maps, core_ids=list(range(NCORES)), **run_kwargs
    )
    outs = [r["out"] for r in res.results]
    full = np.concatenate(outs, axis=0)  # (16, 4096)
    return full.reshape(B, 1, S).astype(np.float32)


# revision 40
# speedup vs baseline: 1.0387x; 1.0387x over previous
# Bass/Tile TRN2 kernel for nn_Attn_2130303779132 (general-score attention).
#
# Math: reference computes
#   proj = einsum('sbh,kh->sbk', enc, W) + b        # (S,B,H) huge matmul
#   energies[b,s] = <hidden[b], proj[s,b]>          # (B,S)
#   out = softmax(energies, axis=-1)
# Algebraically:
#   energies[b,s] = sum_h enc[s,b,h] * v[b,h] + (hidden[b]·bias)
# with v = hidden @ W.  The bias term is constant across s, so softmax
# removes it exactly.
#
# v4 design (fp16 stream, DVE + PE split):
#   * enc is uploaded fp16 (halves HBM traffic; ~16 MiB + 2 MiB W per
#     core is the DMA floor).  The s-range is split per batch:
#     - columns 0..15 (s < 2048): natural layout [s-part, h-free];
#       each E column is one fused DVE scalar_tensor_tensor job
#       (multiply by the broadcast v, accumulate over h).  ~1.13 us/col.
#     - columns 16..31 (s >= 2048): host-transposed layout
#       [h-part, s-free]; each E column is a chain of 8 PE matmuls
#       (lhsT = enc chunk, rhs = v column) accumulating in PSUM.
#       ~1.17 us/col, dominated by instruction dispatch.
#     The two engines run concurrently, each well under the DMA stream.
#   * Tiles are interleaved nat/tr and tapered so the post-DMA tail is
#     only a couple of jobs per engine.
#   * Softmax: partition reductions via PE transpose + ones-matmul;
#     1/Z folded into the transposed output copy.  No gpsimd anywhere.
#
# Sharding: data-parallel over batch. 8 cores x 2 batches each.
# W replicated; no collectives.

import numpy as np

import concourse.bacc as bacc
import concourse.bass as bass
import concourse.bass_isa as bass_isa
import concourse.tile as tile
from concourse import library_config, mybir
from concourse.bass_utils import run_bass_kernel_spmd

S, B, H = 4096, 16, 1024
NCORES = 8
BL = B // NCORES          # local batches per core = 2
P = 128                   # partitions
KC = H // P               # 8 h-chunks
NCHUNK = S // P           # 32 s-chunks of 128
ND = 16                   # DVE columns per batch (natural layout)
NP_ = NCHUNK - ND         # PE columns per batch (transposed layout)
SN = ND * P               # s-extent of the natural half (2048)
# Global DMA tile schedule: (batch, kind, start, count) with
# kind "nat" (col0, ncols) | "tr" (k0, nk).  Both batches' nat tiles
# stream first so the DVE never bubbles mid-kernel; tr tiles are
# tapered so only 16 PE matmuls depend on the final transfer.  PE
# accumulation-group order requires all b0 pairs before b1 pairs.
SCHED = [
    (0, "nat", 0, 10),
    (0, "nat", 10, 4),
    (1, "nat", 0, 10),
    (1, "nat", 10, 4),
    (0, "tr", 0, 3),
    (0, "tr", 3, 4),
    (0, "nat", 14, 2),
    (1, "nat", 14, 2),
    (0, "tr", 7, 1),
    (1, "tr", 0, 3),
    (1, "tr", 3, 4),
    (1, "tr", 7, 1),
]
F32 = mybir.dt.float32
F16 = mybir.dt.float16


def build_bass(loop_n: int = 1) -> bass.Bass:
    """loop_n > 1 wraps the whole kernel body in an on-device For loop —
    used only for steady-state timing (amortizes RPC/launch overhead)."""
    nc = bacc.Bacc("TRN2", target_bir_lowering=False, debug=False,
                   num_devices=NCORES)

    enc_nat = nc.dram_tensor("enc_nat", (BL, SN, H), F16,
                             kind="ExternalInput").ap()
    enc_tr = nc.dram_tensor("enc_tr", (BL, KC, P, S - SN), F16,
                            kind="ExternalInput").ap()
    hid = nc.dram_tensor("hid", (BL, H), F16, kind="ExternalInput").ap()
    w = nc.dram_tensor("w", (H, H), F16, kind="ExternalInput").ap()
    selc = nc.dram_tensor("selc", (BL, BL * P), F16,
                          kind="ExternalInput").ap()
    eye = nc.dram_tensor("eye", (P, P), F32, kind="ExternalInput").ap()
    out = nc.dram_tensor("out", (BL, S), F32, kind="ExternalOutput").ap()

    with tile.TileContext(nc) as tc:
        with (
            tc.tile_pool(name="consts", bufs=1) as consts,
            tc.tile_pool(name="wpool", bufs=1) as wpool,
            tc.tile_pool(name="encpool", bufs=2) as encpool,
            tc.tile_pool(name="scratch", bufs=2) as scratch,
            tc.tile_pool(name="small", bufs=2) as small,
            tc.tile_pool(name="psumc", bufs=1, space="PSUM") as psumc,
            tc.tile_pool(name="psume", bufs=1, space="PSUM") as psume,
            tc.tile_pool(name="psumt", bufs=1, space="PSUM") as psumt,
        ):
            pools = (consts, wpool, encpool, scratch, small,
                     psumc, psume, psumt)

            def body():
                build_body(nc, pools, enc_nat, enc_tr, hid, w, selc, eye,
                           out)

            if loop_n == 1:
                body()
            else:
                with tc.For_i(0, loop_n, 1):
                    body()

    nc.compile()
    return nc


def build_body(nc, pools, enc_nat, enc_tr, hid, w, selc, eye, out):
    consts, wpool, encpool, scratch, small, psumc, psume, psumt = pools

    # Pay the Q7 library IRAM load up front, overlapped with the
    # prologue DMAs — the softmax partition reductions run on gpsimd.
    nc.gpsimd.load_library(library_config.mlp)

    # ---------------- prologue: v = hidden @ W, both layouts ----------
    # The prologue loads go on the SAME sync ring as the enc stream,
    # ahead of it: ring FIFO order guarantees the v-chain inputs (which
    # gate every DVE job) land before the first 2.5 MB enc tile.  Small
    # tensors first so the hidden transposes start immediately.
    ident = consts.tile([P, P], F32, tag="ident")
    nc.sync.dma_start(out=ident, in_=eye)
    hid_sb = consts.tile([BL, H], F16, tag="hid")
    nc.sync.dma_start(out=hid_sb, in_=hid)
    selc_sb = consts.tile([BL, BL * P], F16, tag="selc")
    nc.sync.dma_start(out=selc_sb, in_=selc)
    w_tiles = []
    for i in range(KC):
        w_t = wpool.tile([P, H], F16, tag=f"w{i}", name=f"w{i}")
        nc.sync.dma_start(out=w_t, in_=w[i * P: (i + 1) * P, :])
        w_tiles.append(w_t)

    ident16 = consts.tile([P, P], F16, tag="ident16")
    nc.scalar.copy(out=ident16, in_=ident)
    ones_row = consts.tile([1, P], F32, tag="ones_row")
    nc.vector.memset(ones_row, 1.0)
    ones_col = consts.tile([P, 1], F32, tag="ones_col")
    nc.vector.memset(ones_col, 1.0)
    # Preload the Exp table while the prologue runs so the first real
    # softmax doesn't eat the LoadActFuncSet latency.
    actwarm = consts.tile([1, 1], F32, tag="actwarm")
    nc.scalar.activation(out=actwarm, in_=ones_row[:, 0:1],
                         func=mybir.ActivationFunctionType.Exp)

    # hidden^T via PE transposes: hT[:, 2i + b] = hidden[b, i*128 : ...]
    psum_hT = psumc.tile([P, BL * KC], F16, tag="hT")
    for i in range(KC):
        nc.tensor.transpose(
            out=psum_hT[:, BL * i: BL * i + BL],
            in_=hid_sb[:, i * P: (i + 1) * P],
            identity=ident16[0:BL, 0:BL],
        )
    hT_sb = consts.tile([P, BL * KC], F16, tag="hTsb")
    nc.scalar.copy(out=hT_sb, in_=psum_hT)

    # v = hidden @ W as [BL, H] (v[b,h] = sum_k hid[b,k] W[k,h])
    psum_v = psumc.tile([BL, H], F32, tag="v")
    for j in range(H // 512):
        for i in range(KC):
            nc.tensor.matmul(
                out=psum_v[:, j * 512: (j + 1) * 512],
                lhsT=hT_sb[:, BL * i: BL * i + BL],
                rhs=w_tiles[i][:, j * 512: (j + 1) * 512],
                start=(i == 0),
                stop=(i == KC - 1),
            )
    v_sb16 = consts.tile([BL, H], F16, tag="v16")
    nc.scalar.copy(out=v_sb16, in_=psum_v)

    # vT[:, BL*i + b] = v[b, i*128:(i+1)*128] for the PE path
    psum_vT = psumc.tile([P, BL * KC], F16, tag="vT")
    for i in range(KC):
        nc.tensor.transpose(
            out=psum_vT[:, BL * i: BL * i + BL],
            in_=v_sb16[:, i * P: (i + 1) * P],
            identity=ident16[0:BL, 0:BL],
        )
    vT16 = consts.tile([P, BL * KC], F16, tag="vT16")
    nc.scalar.copy(out=vT16, in_=psum_vT)

    # vb[:, b*H:(b+1)*H] = v[b, :] replicated to all 128 partitions via
    # a selector matmul (selc row b is all-ones over b's 128 columns).
    vb16 = consts.tile([P, BL * H], F16, tag="vb16")
    psum_vb = psumc.tile([P, 512], F32, tag="vb")
    for b in range(BL):
        for j in range(H // 512):
            nc.tensor.matmul(
                out=psum_vb,
                lhsT=selc_sb[:, b * P: (b + 1) * P],
                rhs=v_sb16[:, j * 512: (j + 1) * 512],
                start=True,
                stop=True,
            )
            nc.scalar.copy(
                out=vb16[:, b * H + j * 512: b * H + (j + 1) * 512],
                in_=psum_vb,
            )

    # ---------------- main loop: E columns on DVE and PE --------------
    # E[b][p, c] = sum_h enc[c*128+p, b, h] * v[b, h]
    E = [
        consts.tile([P, NCHUNK], F32, tag=f"E{b}", name=f"E{b}")
        for b in range(BL)
    ]
    # PE half accumulates here; column b*NP_+sc is batch b's col 16+sc.
    psum_E = psume.tile([P, BL * NP_], F32, tag="Epe")
    npairs = NP_ * KC  # PE matmuls per batch
    pair = [0, 0]
    done = [0, 0]      # jobs emitted per batch (nat cols + tr pairs)
    total = ND + npairs
    for b, kind, a0, n in SCHED:
        if kind == "nat":
            et = encpool.tile([P, n, H], F16, tag=f"nat{n}")
            nc.sync.dma_start(
                out=et,
                in_=enc_nat[b, a0 * P: (a0 + n) * P, :].rearrange(
                    "(q p) h -> p q h", p=P
                ),
            )
            for q in range(n):
                c = a0 + q
                prod = scratch.tile([P, H], F16, tag="prod")
                nc.vector.scalar_tensor_tensor(
                    out=prod, in0=et[:, q, :], scalar=1.0,
                    in1=vb16[:, b * H: (b + 1) * H],
                    op0=mybir.AluOpType.mult,
                    op1=mybir.AluOpType.mult,
                    accum_out=E[b][:, c: c + 1],
                )
            done[b] += n
        else:
            et = encpool.tile([P, n, S - SN], F16, tag=f"tr{n}")
            nc.sync.dma_start(
                out=et,
                in_=enc_tr[b, a0: a0 + n].rearrange("k p s -> p k s"),
            )
            for sc in range(NP_):
                for ki in range(n):
                    k = a0 + ki
                    nc.tensor.matmul(
                        out=psum_E[:, b * NP_ + sc: b * NP_ + sc + 1],
                        lhsT=et[:, ki, sc * P: (sc + 1) * P],
                        rhs=vT16[:, BL * k + b: BL * k + b + 1],
                        start=(pair[b] == 0),
                        stop=(pair[b] == npairs - 1),
                    )
                    pair[b] += 1
            done[b] += n * NP_
        if done[b] == total:
            # merge the PE half into E and run this batch's softmax
            nc.scalar.copy(
                out=E[b][:, ND:NCHUNK],
                in_=psum_E[:, b * NP_: (b + 1) * NP_],
            )
            softmax_store(nc, pools, E[b], out, b, ident,
                          ones_row, ones_col)


def softmax_store(nc, pools, Eb, out, b, ident, ones_row, ones_col):
    consts, wpool, encpool, scratch, small, psumc, psume, psumt = pools

    # global max over S: gpsimd all-reduces across partitions in one op
    # (result broadcast to every partition), DVE folds the free dim.
    m_all = small.tile([P, NCHUNK], F32, tag="mall")
    nc.gpsimd.partition_all_reduce(
        out_ap=m_all, in_ap=Eb, channels=P,
        reduce_op=bass_isa.ReduceOp.max,
    )
    negm = small.tile([P, 1], F32, tag="negm")
    nc.vector.tensor_reduce(
        out=negm, in_=m_all, axis=mybir.AxisListType.X,
        op=mybir.AluOpType.max, negate=True,
    )

    eexp = small.tile([P, NCHUNK], F32, tag="eexp")
    nc.scalar.activation(
        out=eexp, in_=Eb,
        func=mybir.ActivationFunctionType.Exp,
        bias=negm, scale=1.0,
    )
    # transpose the unnormalized numerator early; the 1/Z scale lands in
    # the final ACT copy as a per-partition scalar.
    psum_eT = psumt.tile([NCHUNK, P], F32, tag="pT", name="psum_eT")
    nc.tensor.transpose(out=psum_eT, in_=eexp, identity=ident)

    # total = sum over S: free-dim sum on DVE, partition sum on gpsimd
    # (already broadcast), reciprocal — every partition holds 1/Z.
    rowsum = small.tile([P, 1], F32, tag="rowsum")
    nc.vector.tensor_reduce(
        out=rowsum, in_=eexp, axis=mybir.AxisListType.X,
        op=mybir.AluOpType.add,
    )
    tot = small.tile([P, 1], F32, tag="tot")
    nc.gpsimd.partition_all_reduce(
        out_ap=tot, in_ap=rowsum, channels=P,
        reduce_op=bass_isa.ReduceOp.add,
    )
    rtot = small.tile([P, 1], F32, tag="rtot")
    nc.vector.reciprocal(out=rtot, in_=tot)

    # out[b][sc*128 + p] = eexp[p, sc] / Z: scaled copy of the transpose
    pT_sb = small.tile([NCHUNK, P], F32, tag="pTsb")
    nc.scalar.activation(
        out=pT_sb, in_=psum_eT,
        func=mybir.ActivationFunctionType.Copy,
        scale=rtot[0:NCHUNK, :],
    )
    nc.scalar.dma_start(
        out=out[b].rearrange("(c p) -> c p", p=P), in_=pT_sb
    )


_NC_CACHE = None


def _get_nc() -> bass.Bass:
    global _NC_CACHE
    if _NC_CACHE is None:
        _NC_CACHE = build_bass()
    return _NC_CACHE


def make_in_maps(hidden, encoder_outputs, W):
    hidden = np.asarray(hidden, dtype=np.float32)
    encoder_outputs = np.asarray(encoder_outputs, dtype=np.float32)
    W = np.asarray(W, dtype=np.float32)
    # natural half (s < SN): (SN, B, H) -> (B, SN, H) fp16
    enc_nat = np.ascontiguousarray(
        encoder_outputs[:SN].transpose(1, 0, 2)
    ).astype(np.float16)
    # transposed half (s >= SN): (S-SN, B, H) -> (B, H, S-SN) fp16
    enc_tr = np.ascontiguousarray(
        encoder_outputs[SN:].transpose(1, 2, 0)
    ).astype(np.float16)
    w16 = np.ascontiguousarray(W).astype(np.float16)
    hid16 = hidden[0].astype(np.float16)
    eye = np.eye(P, dtype=np.float32)
    selc = np.zeros((BL, BL * P), dtype=np.float16)
    for b in range(BL):
        selc[b, b * P: (b + 1) * P] = 1.0
    in_maps = []
    for c in range(NCORES):
        in_maps.append(
            {
                "enc_nat": enc_nat[c * BL: (c + 1) * BL],
                "enc_tr": enc_tr[c * BL: (c + 1) * BL].reshape(
                    BL, KC, P, S - SN
                ),
                "hid": np.ascontiguousarray(hid16[c * BL: (c + 1) * BL]),
                "w": w16,
                "selc": selc,
                "eye": eye,
            }
        )
    return in_maps


def kernel(hidden, encoder_outputs, W, b, **run_kwargs):
    # `b` (the nn.Linear bias) shifts every energy row by a per-batch
    # constant, which softmax cancels exactly — unused on device.
    nc = _get_nc()
    in_maps = make_in_maps(hidden, encoder_outputs, W)
    res = run_bass_kernel_spmd(
        nc, in_maps, core_ids=list(range(NCORES)), **run_kwargs
    )
    outs = [r["out"] for r in res.results]
    full = np.concatenate(outs, axis=0)  # (16, 4096)
    return full.reshape(B, 1, S).astype(np.float32)


# revision 42
# speedup vs baseline: 1.0491x; 1.0100x over previous
# Bass/Tile TRN2 kernel for nn_Attn_2130303779132 (general-score attention).
#
# Math: reference computes
#   proj = einsum('sbh,kh->sbk', enc, W) + b        # (S,B,H) huge matmul
#   energies[b,s] = <hidden[b], proj[s,b]>          # (B,S)
#   out = softmax(energies, axis=-1)
# Algebraically:
#   energies[b,s] = sum_h enc[s,b,h] * v[b,h] + (hidden[b]·bias)
# with v = hidden @ W.  The bias term is constant across s, so softmax
# removes it exactly.
#
# v4 design (fp16 stream, DVE + PE split):
#   * enc is uploaded fp16 (halves HBM traffic; ~16 MiB + 2 MiB W per
#     core is the DMA floor).  The s-range is split per batch:
#     - columns 0..15 (s < 2048): natural layout [s-part, h-free];
#       each E column is one fused DVE scalar_tensor_tensor job
#       (multiply by the broadcast v, accumulate over h).  ~1.13 us/col.
#     - columns 16..31 (s >= 2048): host-transposed layout
#       [h-part, s-free]; each E column is a chain of 8 PE matmuls
#       (lhsT = enc chunk, rhs = v column) accumulating in PSUM.
#       ~1.17 us/col, dominated by instruction dispatch.
#     The two engines run concurrently, each well under the DMA stream.
#   * Tiles are interleaved nat/tr and tapered so the post-DMA tail is
#     only a couple of jobs per engine.
#   * Softmax: partition reductions via PE transpose + ones-matmul;
#     1/Z folded into the transposed output copy.  No gpsimd anywhere.
#
# Sharding: data-parallel over batch. 8 cores x 2 batches each.
# W replicated; no collectives.

import numpy as np

import concourse.bacc as bacc
import concourse.bass as bass
import concourse.bass_isa as bass_isa
import concourse.tile as tile
from concourse import library_config, mybir
from concourse.bass_utils import run_bass_kernel_spmd

S, B, H = 4096, 16, 1024
NCORES = 8
BL = B // NCORES          # local batches per core = 2
P = 128                   # partitions
KC = H // P               # 8 h-chunks
NCHUNK = S // P           # 32 s-chunks of 128
ND = 16                   # DVE columns per batch (natural layout)
NP_ = NCHUNK - ND         # PE columns per batch (transposed layout)
SN = ND * P               # s-extent of the natural half (2048)
# Global DMA tile schedule: (batch, kind, start, count) with
# kind "nat" (col0, ncols) | "tr" (k0, nk).  Both batches' nat tiles
# stream first so the DVE never bubbles mid-kernel; tr tiles are
# tapered so only 16 PE matmuls depend on the final transfer.  PE
# accumulation-group order requires all b0 pairs before b1 pairs.
SCHED = [
    (0, "nat", 0, 10),
    (0, "nat", 10, 4),
    (1, "nat", 0, 10),
    (1, "nat", 10, 4),
    (0, "tr", 0, 3),
    (0, "tr", 3, 4),
    (0, "nat", 14, 2),
    (1, "nat", 14, 2),
    (0, "tr", 7, 1),
    (1, "tr", 0, 3),
    (1, "tr", 3, 4),
    (1, "tr", 7, 1),
]
F32 = mybir.dt.float32
F16 = mybir.dt.float16


def build_bass(loop_n: int = 1) -> bass.Bass:
    """loop_n > 1 wraps the whole kernel body in an on-device For loop —
    used only for steady-state timing (amortizes RPC/launch overhead)."""
    nc = bacc.Bacc("TRN2", target_bir_lowering=False, debug=False,
                   num_devices=NCORES)

    enc_nat = nc.dram_tensor("enc_nat", (BL, SN, H), F16,
                             kind="ExternalInput").ap()
    enc_tr = nc.dram_tensor("enc_tr", (BL, KC, P, S - SN), F16,
                            kind="ExternalInput").ap()
    hid = nc.dram_tensor("hid", (BL, H), F16, kind="ExternalInput").ap()
    w = nc.dram_tensor("w", (H, H), F16, kind="ExternalInput").ap()
    selc = nc.dram_tensor("selc", (BL, BL * P), F16,
                          kind="ExternalInput").ap()
    eye = nc.dram_tensor("eye", (P, P), F32, kind="ExternalInput").ap()
    out = nc.dram_tensor("out", (BL, S), F32, kind="ExternalOutput").ap()

    with tile.TileContext(nc) as tc:
        with (
            tc.tile_pool(name="consts", bufs=1) as consts,
            tc.tile_pool(name="wpool", bufs=1) as wpool,
            tc.tile_pool(name="encpool", bufs=2) as encpool,
            tc.tile_pool(name="scratch", bufs=2) as scratch,
            tc.tile_pool(name="small", bufs=2) as small,
            tc.tile_pool(name="psumc", bufs=1, space="PSUM") as psumc,
            tc.tile_pool(name="psume", bufs=1, space="PSUM") as psume,
            tc.tile_pool(name="psumt", bufs=1, space="PSUM") as psumt,
        ):
            pools = (consts, wpool, encpool, scratch, small,
                     psumc, psume, psumt)

            def body():
                build_body(nc, pools, enc_nat, enc_tr, hid, w, selc, eye,
                           out)

            if loop_n == 1:
                body()
            else:
                with tc.For_i(0, loop_n, 1):
                    body()

    nc.compile()
    return nc


def build_body(nc, pools, enc_nat, enc_tr, hid, w, selc, eye, out):
    consts, wpool, encpool, scratch, small, psumc, psume, psumt = pools

    # Pay the Q7 library IRAM load up front, overlapped with the
    # prologue DMAs — the softmax partition reductions run on gpsimd.
    nc.gpsimd.load_library(library_config.mlp)

    # ---------------- prologue: v = hidden @ W, both layouts ----------
    # The prologue loads go on the SAME sync ring as the enc stream,
    # ahead of it: ring FIFO order guarantees the v-chain inputs (which
    # gate every DVE job) land before the first 2.5 MB enc tile.  W
    # first: the small tensors' descriptor generation then hides under
    # the W transfers instead of serializing ahead of them.
    ident = consts.tile([P, P], F32, tag="ident")
    nc.sync.dma_start(out=ident, in_=eye)
    hid_sb = consts.tile([BL, H], F16, tag="hid")
    nc.sync.dma_start(out=hid_sb, in_=hid)
    w_tiles = []
    for i in range(KC):
        w_t = wpool.tile([P, H], F16, tag=f"w{i}", name=f"w{i}")
        nc.sync.dma_start(out=w_t, in_=w[i * P: (i + 1) * P, :])
        w_tiles.append(w_t)
    # selc is only needed at the very end of the v-chain (vb selector):
    # its descriptor generation hides under the W transfers.
    selc_sb = consts.tile([BL, BL * P], F16, tag="selc")
    nc.sync.dma_start(out=selc_sb, in_=selc)

    ident16 = consts.tile([P, P], F16, tag="ident16")
    nc.scalar.copy(out=ident16, in_=ident)
    ones_row = consts.tile([1, P], F32, tag="ones_row")
    nc.vector.memset(ones_row, 1.0)
    ones_col = consts.tile([P, 1], F32, tag="ones_col")
    nc.vector.memset(ones_col, 1.0)
    # Preload the Exp table while the prologue runs so the first real
    # softmax doesn't eat the LoadActFuncSet latency.
    actwarm = consts.tile([1, 1], F32, tag="actwarm")
    nc.scalar.activation(out=actwarm, in_=ones_row[:, 0:1],
                         func=mybir.ActivationFunctionType.Exp)

    # hidden^T via PE transposes: hT[:, 2i + b] = hidden[b, i*128 : ...]
    psum_hT = psumc.tile([P, BL * KC], F16, tag="hT")
    for i in range(KC):
        nc.tensor.transpose(
            out=psum_hT[:, BL * i: BL * i + BL],
            in_=hid_sb[:, i * P: (i + 1) * P],
            identity=ident16[0:BL, 0:BL],
        )
    hT_sb = consts.tile([P, BL * KC], F16, tag="hTsb")
    nc.scalar.copy(out=hT_sb, in_=psum_hT)

    # v = hidden @ W as [BL, H] (v[b,h] = sum_k hid[b,k] W[k,h])
    psum_v = psumc.tile([BL, H], F32, tag="v")
    for j in range(H // 512):
        for i in range(KC):
            nc.tensor.matmul(
                out=psum_v[:, j * 512: (j + 1) * 512],
                lhsT=hT_sb[:, BL * i: BL * i + BL],
                rhs=w_tiles[i][:, j * 512: (j + 1) * 512],
                start=(i == 0),
                stop=(i == KC - 1),
            )
    v_sb16 = consts.tile([BL, H], F16, tag="v16")
    nc.scalar.copy(out=v_sb16, in_=psum_v)

    # vT[:, BL*i + b] = v[b, i*128:(i+1)*128] for the PE path
    psum_vT = psumc.tile([P, BL * KC], F16, tag="vT")
    for i in range(KC):
        nc.tensor.transpose(
            out=psum_vT[:, BL * i: BL * i + BL],
            in_=v_sb16[:, i * P: (i + 1) * P],
            identity=ident16[0:BL, 0:BL],
        )
    vT16 = consts.tile([P, BL * KC], F16, tag="vT16")
    nc.scalar.copy(out=vT16, in_=psum_vT)

    # vb[:, b*H:(b+1)*H] = v[b, :] replicated to all 128 partitions via
    # a selector matmul (selc row b is all-ones over b's 128 columns).
    vb16 = consts.tile([P, BL * H], F16, tag="vb16")
    psum_vb = psumc.tile([P, 512], F32, tag="vb")
    for b in range(BL):
        for j in range(H // 512):
            nc.tensor.matmul(
                out=psum_vb,
                lhsT=selc_sb[:, b * P: (b + 1) * P],
                rhs=v_sb16[:, j * 512: (j + 1) * 512],
                start=True,
                stop=True,
            )
            nc.scalar.copy(
                out=vb16[:, b * H + j * 512: b * H + (j + 1) * 512],
                in_=psum_vb,
            )

    # ---------------- main loop: E columns on DVE and PE --------------
    # E[b][p, c] = sum_h enc[c*128+p, b, h] * v[b, h]
    E = [
        consts.tile([P, NCHUNK], F32, tag=f"E{b}", name=f"E{b}")
        for b in range(BL)
    ]
    # PE half accumulates here; column b*NP_+sc is batch b's col 16+sc.
    psum_E = psume.tile([P, BL * NP_], F32, tag="Epe")
    npairs = NP_ * KC  # PE matmuls per batch
    pair = [0, 0]
    done = [0, 0]      # jobs emitted per batch (nat cols + tr pairs)
    total = ND + npairs
    for b, kind, a0, n in SCHED:
        if kind == "nat":
            et = encpool.tile([P, n, H], F16, tag=f"nat{n}")
            nc.sync.dma_start(
                out=et,
                in_=enc_nat[b, a0 * P: (a0 + n) * P, :].rearrange(
                    "(q p) h -> p q h", p=P
                ),
            )
            for q in range(n):
                c = a0 + q
                prod = scratch.tile([P, H], F16, tag="prod")
                nc.vector.scalar_tensor_tensor(
                    out=prod, in0=et[:, q, :], scalar=1.0,
                    in1=vb16[:, b * H: (b + 1) * H],
                    op0=mybir.AluOpType.mult,
                    op1=mybir.AluOpType.mult,
                    accum_out=E[b][:, c: c + 1],
                )
            done[b] += n
        else:
            et = encpool.tile([P, n, S - SN], F16, tag=f"tr{n}")
            nc.sync.dma_start(
                out=et,
                in_=enc_tr[b, a0: a0 + n].rearrange("k p s -> p k s"),
            )
            for sc in range(NP_):
                for ki in range(n):
                    k = a0 + ki
                    nc.tensor.matmul(
                        out=psum_E[:, b * NP_ + sc: b * NP_ + sc + 1],
                        lhsT=et[:, ki, sc * P: (sc + 1) * P],
                        rhs=vT16[:, BL * k + b: BL * k + b + 1],
                        start=(pair[b] == 0),
                        stop=(pair[b] == npairs - 1),
                    )
                    pair[b] += 1
            done[b] += n * NP_
        if done[b] == total:
            # merge the PE half into E and run this batch's softmax
            nc.scalar.copy(
                out=E[b][:, ND:NCHUNK],
                in_=psum_E[:, b * NP_: (b + 1) * NP_],
            )
            softmax_store(nc, pools, E[b], out, b, ident,
                          ones_row, ones_col)


def softmax_store(nc, pools, Eb, out, b, ident, ones_row, ones_col):
    consts, wpool, encpool, scratch, small, psumc, psume, psumt = pools

    # global max over S: gpsimd all-reduces across partitions in one op
    # (result broadcast to every partition), DVE folds the free dim.
    m_all = small.tile([P, NCHUNK], F32, tag="mall")
    nc.gpsimd.partition_all_reduce(
        out_ap=m_all, in_ap=Eb, channels=P,
        reduce_op=bass_isa.ReduceOp.max,
    )
    negm = small.tile([P, 1], F32, tag="negm")
    nc.vector.tensor_reduce(
        out=negm, in_=m_all, axis=mybir.AxisListType.X,
        op=mybir.AluOpType.max, negate=True,
    )

    eexp = small.tile([P, NCHUNK], F32, tag="eexp")
    nc.scalar.activation(
        out=eexp, in_=Eb,
        func=mybir.ActivationFunctionType.Exp,
        bias=negm, scale=1.0,
    )
    # transpose the unnormalized numerator early; the 1/Z scale lands in
    # the final ACT copy as a per-partition scalar.
    psum_eT = psumt.tile([NCHUNK, P], F32, tag="pT", name="psum_eT")
    nc.tensor.transpose(out=psum_eT, in_=eexp, identity=ident)

    # total = sum over S: free-dim sum on DVE, partition sum on gpsimd
    # (already broadcast), reciprocal — every partition holds 1/Z.
    rowsum = small.tile([P, 1], F32, tag="rowsum")
    nc.vector.tensor_reduce(
        out=rowsum, in_=eexp, axis=mybir.AxisListType.X,
        op=mybir.AluOpType.add,
    )
    tot = small.tile([P, 1], F32, tag="tot")
    nc.gpsimd.partition_all_reduce(
        out_ap=tot, in_ap=rowsum, channels=P,
        reduce_op=bass_isa.ReduceOp.add,
    )
    rtot = small.tile([P, 1], F32, tag="rtot")
    nc.vector.reciprocal(out=rtot, in_=tot)

    # out[b][sc*128 + p] = eexp[p, sc] / Z: scaled copy of the transpose
    pT_sb = small.tile([NCHUNK, P], F32, tag="pTsb")
    nc.scalar.activation(
        out=pT_sb, in_=psum_eT,
        func=mybir.ActivationFunctionType.Copy,
        scale=rtot[0:NCHUNK, :],
    )
    nc.scalar.dma_start(
        out=out[b].rearrange("(c p) -> c p", p=P), in_=pT_sb
    )


_NC_CACHE = None


def _get_nc() -> bass.Bass:
    global _NC_CACHE
    if _NC_CACHE is None:
        _NC_CACHE = build_bass()
    return _NC_CACHE


def make_in_maps(hidden, encoder_outputs, W):
    hidden = np.asarray(hidden, dtype=np.float32)
    encoder_outputs = np.asarray(encoder_outputs, dtype=np.float32)
    W = np.asarray(W, dtype=np.float32)
    # natural half (s < SN): (SN, B, H) -> (B, SN, H) fp16
    enc_nat = np.ascontiguousarray(
        encoder_outputs[:SN].transpose(1, 0, 2)
    ).astype(np.float16)
    # transposed half (s >= SN): (S-SN, B, H) -> (B, H, S-SN) fp16
    enc_tr = np.ascontiguousarray(
        encoder_outputs[SN:].transpose(1, 2, 0)
    ).astype(np.float16)
    w16 = np.ascontiguousarray(W).astype(np.float16)
    hid16 = hidden[0].astype(np.float16)
    eye = np.eye(P, dtype=np.float32)
    selc = np.zeros((BL, BL * P), dtype=np.float16)
    for b in range(BL):
        selc[b, b * P: (b + 1) * P] = 1.0
    in_maps = []
    for c in range(NCORES):
        in_maps.append(
            {
                "enc_nat": enc_nat[c * BL: (c + 1) * BL],
                "enc_tr": enc_tr[c * BL: (c + 1) * BL].reshape(
                    BL, KC, P, S - SN
                ),
                "hid": np.ascontiguousarray(hid16[c * BL: (c + 1) * BL]),
                "w": w16,
                "selc": selc,
                "eye": eye,
            }
        )
    return in_maps


def kernel(hidden, encoder_outputs, W, b, **run_kwargs):
    # `b` (the nn.Linear bias) shifts every energy row by a per-batch
    # constant, which softmax cancels exactly — unused on device.
    nc = _get_nc()
    in_maps = make_in_maps(hidden, encoder_outputs, W)
    res = run_bass_kernel_spmd(
        nc, in_maps, core_ids=list(range(NCORES)), **run_kwargs
    )
    outs = [r["out"] for r in res.results]
    full = np.concatenate(outs, axis=0)  # (16, 4096)
    return full.reshape(B, 1, S).astype(np.float32)


# revision 49
# speedup vs baseline: 1.0501x; 1.0009x over previous
# Bass/Tile TRN2 kernel for nn_Attn_2130303779132 (general-score attention).
#
# Math: reference computes
#   proj = einsum('sbh,kh->sbk', enc, W) + b        # (S,B,H) huge matmul
#   energies[b,s] = <hidden[b], proj[s,b]>          # (B,S)
#   out = softmax(energies, axis=-1)
# Algebraically:
#   energies[b,s] = sum_h enc[s,b,h] * v[b,h] + (hidden[b]·bias)
# with v = hidden @ W.  The bias term is constant across s, so softmax
# removes it exactly.
#
# v4 design (fp16 stream, DVE + PE split):
#   * enc is uploaded fp16 (halves HBM traffic; ~16 MiB + 2 MiB W per
#     core is the DMA floor).  The s-range is split per batch:
#     - columns 0..15 (s < 2048): natural layout [s-part, h-free];
#       each E column is one fused DVE scalar_tensor_tensor job
#       (multiply by the broadcast v, accumulate over h).  ~1.13 us/col.
#     - columns 16..31 (s >= 2048): host-transposed layout
#       [h-part, s-free]; each E column is a chain of 8 PE matmuls
#       (lhsT = enc chunk, rhs = v column) accumulating in PSUM.
#       ~1.17 us/col, dominated by instruction dispatch.
#     The two engines run concurrently, each well under the DMA stream.
#   * Tiles are interleaved nat/tr and tapered so the post-DMA tail is
#     only a couple of jobs per engine.
#   * Softmax: partition reductions via PE transpose + ones-matmul;
#     1/Z folded into the transposed output copy.  No gpsimd anywhere.
#
# Sharding: data-parallel over batch. 8 cores x 2 batches each.
# W replicated; no collectives.

import numpy as np

import concourse.bacc as bacc
import concourse.bass as bass
import concourse.bass_isa as bass_isa
import concourse.tile as tile
from concourse import library_config, mybir
from concourse.bass_utils import run_bass_kernel_spmd

S, B, H = 4096, 16, 1024
NCORES = 8
BL = B // NCORES          # local batches per core = 2
P = 128                   # partitions
KC = H // P               # 8 h-chunks
NCHUNK = S // P           # 32 s-chunks of 128
ND = 16                   # DVE columns per batch (natural layout)
NP_ = NCHUNK - ND         # PE columns per batch (transposed layout)
SN = ND * P               # s-extent of the natural half (2048)
# Global DMA tile schedule: (batch, kind, start, count) with
# kind "nat" (col0, ncols) | "tr" (k0, nk).  Both batches' nat tiles
# stream first so the DVE never bubbles mid-kernel; tr tiles are
# tapered so only 16 PE matmuls depend on the final transfer.  PE
# accumulation-group order requires all b0 pairs before b1 pairs.
SCHED = [
    (0, "nat", 0, 10),
    (0, "nat", 10, 4),
    (1, "nat", 0, 10),
    (1, "nat", 10, 4),
    (0, "tr", 0, 3),
    (0, "tr", 3, 4),
    (0, "nat", 14, 2),
    (1, "nat", 14, 2),
    (0, "tr", 7, 1),
    (1, "tr", 0, 3),
    (1, "tr", 3, 4),
    (1, "trh", 7, 1),   # k7 for PE columns 0..7
    (1, "trh2", 7, 1),  # k7 for PE columns 8..15: only 8 pairs tail
]
F32 = mybir.dt.float32
F16 = mybir.dt.float16


def build_bass(loop_n: int = 1) -> bass.Bass:
    """loop_n > 1 wraps the whole kernel body in an on-device For loop —
    used only for steady-state timing (amortizes RPC/launch overhead)."""
    nc = bacc.Bacc("TRN2", target_bir_lowering=False, debug=False,
                   num_devices=NCORES)

    enc_nat = nc.dram_tensor("enc_nat", (BL, SN, H), F16,
                             kind="ExternalInput").ap()
    enc_tr = nc.dram_tensor("enc_tr", (BL, KC, P, S - SN), F16,
                            kind="ExternalInput").ap()
    # hid and selc packed into one upload (one descriptor generation)
    aux = nc.dram_tensor("aux", (BL, H + BL * P), F16,
                         kind="ExternalInput").ap()
    w = nc.dram_tensor("w", (H, H), F16, kind="ExternalInput").ap()
    eye = nc.dram_tensor("eye", (P, P), F32, kind="ExternalInput").ap()
    out = nc.dram_tensor("out", (BL, S), F32, kind="ExternalOutput").ap()

    with tile.TileContext(nc) as tc:
        with (
            tc.tile_pool(name="consts", bufs=1) as consts,
            tc.tile_pool(name="wpool", bufs=1) as wpool,
            tc.tile_pool(name="encpool", bufs=2) as encpool,
            tc.tile_pool(name="scratch", bufs=2) as scratch,
            tc.tile_pool(name="small", bufs=2) as small,
            tc.tile_pool(name="psumc", bufs=1, space="PSUM") as psumc,
            tc.tile_pool(name="psume", bufs=1, space="PSUM") as psume,
            tc.tile_pool(name="psumt", bufs=1, space="PSUM") as psumt,
        ):
            pools = (consts, wpool, encpool, scratch, small,
                     psumc, psume, psumt)

            def body():
                build_body(nc, pools, enc_nat, enc_tr, aux, w, eye, out)

            if loop_n == 1:
                body()
            else:
                with tc.For_i(0, loop_n, 1):
                    body()

    nc.compile()
    return nc


def build_body(nc, pools, enc_nat, enc_tr, aux, w, eye, out):
    consts, wpool, encpool, scratch, small, psumc, psume, psumt = pools

    # Pay the Q7 library IRAM load up front, overlapped with the
    # prologue DMAs — the softmax partition reductions run on gpsimd.
    nc.gpsimd.load_library(library_config.mlp)

    # ---------------- prologue: v = hidden @ W, both layouts ----------
    # The prologue loads go on the SAME sync ring as the enc stream,
    # ahead of it: ring FIFO order guarantees the v-chain inputs (which
    # gate every DVE job) land before the first 2.5 MB enc tile.  W
    # first: the small tensors' descriptor generation then hides under
    # the W transfers instead of serializing ahead of them.
    ident = consts.tile([P, P], F32, tag="ident")
    nc.sync.dma_start(out=ident, in_=eye)
    aux_sb = consts.tile([BL, H + BL * P], F16, tag="aux")
    nc.sync.dma_start(out=aux_sb, in_=aux)
    hid_sb = aux_sb[:, 0:H]
    selc_sb = aux_sb[:, H: H + BL * P]
    w_tiles = []
    for i in range(KC):
        w_t = wpool.tile([P, H], F16, tag=f"w{i}", name=f"w{i}")
        nc.sync.dma_start(out=w_t, in_=w[i * P: (i + 1) * P, :])
        w_tiles.append(w_t)

    ident16 = consts.tile([P, P], F16, tag="ident16")
    nc.scalar.copy(out=ident16, in_=ident)
    ones_row = consts.tile([1, P], F32, tag="ones_row")
    nc.vector.memset(ones_row, 1.0)
    ones_col = consts.tile([P, 1], F32, tag="ones_col")
    nc.vector.memset(ones_col, 1.0)
    # Preload the Exp table while the prologue runs so the first real
    # softmax doesn't eat the LoadActFuncSet latency.
    actwarm = consts.tile([1, 1], F32, tag="actwarm")
    nc.scalar.activation(out=actwarm, in_=ones_row[:, 0:1],
                         func=mybir.ActivationFunctionType.Exp)

    # hidden^T via PE transposes: hT[:, 2i + b] = hidden[b, i*128 : ...]
    psum_hT = psumc.tile([P, BL * KC], F16, tag="hT")
    for i in range(KC):
        nc.tensor.transpose(
            out=psum_hT[:, BL * i: BL * i + BL],
            in_=hid_sb[:, i * P: (i + 1) * P],
            identity=ident16[0:BL, 0:BL],
        )
    hT_sb = consts.tile([P, BL * KC], F16, tag="hTsb")
    nc.scalar.copy(out=hT_sb, in_=psum_hT)

    # v = hidden @ W as [BL, H] (v[b,h] = sum_k hid[b,k] W[k,h])
    psum_v = psumc.tile([BL, H], F32, tag="v")
    for j in range(H // 512):
        for i in range(KC):
            nc.tensor.matmul(
                out=psum_v[:, j * 512: (j + 1) * 512],
                lhsT=hT_sb[:, BL * i: BL * i + BL],
                rhs=w_tiles[i][:, j * 512: (j + 1) * 512],
                start=(i == 0),
                stop=(i == KC - 1),
            )
    v_sb16 = consts.tile([BL, H], F16, tag="v16")
    nc.scalar.copy(out=v_sb16, in_=psum_v)

    # vT[:, BL*i + b] = v[b, i*128:(i+1)*128] for the PE path
    psum_vT = psumc.tile([P, BL * KC], F16, tag="vT")
    for i in range(KC):
        nc.tensor.transpose(
            out=psum_vT[:, BL * i: BL * i + BL],
            in_=v_sb16[:, i * P: (i + 1) * P],
            identity=ident16[0:BL, 0:BL],
        )
    vT16 = consts.tile([P, BL * KC], F16, tag="vT16")
    nc.scalar.copy(out=vT16, in_=psum_vT)

    # vb[:, b*H:(b+1)*H] = v[b, :] replicated to all 128 partitions via
    # a selector matmul (selc row b is all-ones over b's 128 columns).
    vb16 = consts.tile([P, BL * H], F16, tag="vb16")
    psum_vb = psumc.tile([P, 512], F32, tag="vb")
    for b in range(BL):
        for j in range(H // 512):
            nc.tensor.matmul(
                out=psum_vb,
                lhsT=selc_sb[:, b * P: (b + 1) * P],
                rhs=v_sb16[:, j * 512: (j + 1) * 512],
                start=True,
                stop=True,
            )
            nc.scalar.copy(
                out=vb16[:, b * H + j * 512: b * H + (j + 1) * 512],
                in_=psum_vb,
            )

    # ---------------- main loop: E columns on DVE and PE --------------
    # E[b][p, c] = sum_h enc[c*128+p, b, h] * v[b, h]
    E = [
        consts.tile([P, NCHUNK], F32, tag=f"E{b}", name=f"E{b}")
        for b in range(BL)
    ]
    # PE half accumulates here; column b*NP_+sc is batch b's col 16+sc.
    psum_E = psume.tile([P, BL * NP_], F32, tag="Epe")
    npairs = NP_ * KC  # PE matmuls per batch
    pair = [0, 0]
    done = [0, 0]      # jobs emitted per batch (nat cols + tr pairs)
    total = ND + npairs
    for b, kind, a0, n in SCHED:
        if kind == "nat":
            et = encpool.tile([P, n, H], F16, tag=f"nat{n}")
            nc.sync.dma_start(
                out=et,
                in_=enc_nat[b, a0 * P: (a0 + n) * P, :].rearrange(
                    "(q p) h -> p q h", p=P
                ),
            )
            for q in range(n):
                c = a0 + q
                prod = scratch.tile([P, H], F16, tag="prod")
                nc.vector.scalar_tensor_tensor(
                    out=prod, in0=et[:, q, :], scalar=1.0,
                    in1=vb16[:, b * H: (b + 1) * H],
                    op0=mybir.AluOpType.mult,
                    op1=mybir.AluOpType.mult,
                    accum_out=E[b][:, c: c + 1],
                )
            done[b] += n
        else:
            # "tr": all NP_ columns for nk chunks; "trh"/"trh2": one
            # chunk, first/second half of the columns (smaller tail).
            half = {"tr": None, "trh": 0, "trh2": 1}[kind]
            scs = (range(NP_) if half is None
                   else range(half * (NP_ // 2), (half + 1) * (NP_ // 2)))
            s0, s1 = scs[0] * P, (scs[-1] + 1) * P
            et = encpool.tile([P, n, s1 - s0], F16, tag=f"{kind}{n}")
            nc.sync.dma_start(
                out=et,
                in_=enc_tr[b, a0: a0 + n, :, s0:s1].rearrange(
                    "k p s -> p k s"
                ),
            )
            for sc in scs:
                for ki in range(n):
                    k = a0 + ki
                    nc.tensor.matmul(
                        out=psum_E[:, b * NP_ + sc: b * NP_ + sc + 1],
                        lhsT=et[:, ki, (sc - scs[0]) * P: (sc - scs[0] + 1) * P],
                        rhs=vT16[:, BL * k + b: BL * k + b + 1],
                        start=(pair[b] == 0),
                        stop=(pair[b] == npairs - 1),
                    )
                    pair[b] += 1
            done[b] += n * len(scs)
        if done[b] == total:
            # merge the PE half into E and run this batch's softmax
            nc.scalar.copy(
                out=E[b][:, ND:NCHUNK],
                in_=psum_E[:, b * NP_: (b + 1) * NP_],
            )
            softmax_store(nc, pools, E[b], out, b, ident,
                          ones_row, ones_col)


def softmax_store(nc, pools, Eb, out, b, ident, ones_row, ones_col):
    consts, wpool, encpool, scratch, small, psumc, psume, psumt = pools

    # global max over S: gpsimd all-reduces across partitions in one op
    # (result broadcast to every partition), DVE folds the free dim.
    m_all = small.tile([P, NCHUNK], F32, tag="mall")
    nc.gpsimd.partition_all_reduce(
        out_ap=m_all, in_ap=Eb, channels=P,
        reduce_op=bass_isa.ReduceOp.max,
    )
    negm = small.tile([P, 1], F32, tag="negm")
    nc.vector.tensor_reduce(
        out=negm, in_=m_all, axis=mybir.AxisListType.X,
        op=mybir.AluOpType.max, negate=True,
    )

    eexp = small.tile([P, NCHUNK], F32, tag="eexp")
    nc.scalar.activation(
        out=eexp, in_=Eb,
        func=mybir.ActivationFunctionType.Exp,
        bias=negm, scale=1.0,
    )
    # transpose the unnormalized numerator early; the 1/Z scale lands in
    # the final ACT copy as a per-partition scalar.
    psum_eT = psumt.tile([NCHUNK, P], F32, tag="pT", name="psum_eT")
    nc.tensor.transpose(out=psum_eT, in_=eexp, identity=ident)

    # total = sum over S: free-dim sum on DVE, partition sum on gpsimd
    # (already broadcast), reciprocal — every partition holds 1/Z.
    rowsum = small.tile([P, 1], F32, tag="rowsum")
    nc.vector.tensor_reduce(
        out=rowsum, in_=eexp, axis=mybir.AxisListType.X,
        op=mybir.AluOpType.add,
    )
    tot = small.tile([P, 1], F32, tag="tot")
    nc.gpsimd.partition_all_reduce(
        out_ap=tot, in_ap=rowsum, channels=P,
        reduce_op=bass_isa.ReduceOp.add,
    )
    rtot = small.tile([P, 1], F32, tag="rtot")
    nc.vector.reciprocal(out=rtot, in_=tot)

    # out[b][sc*128 + p] = eexp[p, sc] / Z: scaled copy of the transpose
    pT_sb = small.tile([NCHUNK, P], F32, tag="pTsb")
    nc.scalar.activation(
        out=pT_sb, in_=psum_eT,
        func=mybir.ActivationFunctionType.Copy,
        scale=rtot[0:NCHUNK, :],
    )
    nc.scalar.dma_start(
        out=out[b].rearrange("(c p) -> c p", p=P), in_=pT_sb
    )


_NC_CACHE = None


def _get_nc() -> bass.Bass:
    global _NC_CACHE
    if _NC_CACHE is None:
        _NC_CACHE = build_bass()
    return _NC_CACHE


def make_in_maps(hidden, encoder_outputs, W):
    hidden = np.asarray(hidden, dtype=np.float32)
    encoder_outputs = np.asarray(encoder_outputs, dtype=np.float32)
    W = np.asarray(W, dtype=np.float32)
    # natural half (s < SN): (SN, B, H) -> (B, SN, H) fp16
    enc_nat = np.ascontiguousarray(
        encoder_outputs[:SN].transpose(1, 0, 2)
    ).astype(np.float16)
    # transposed half (s >= SN): (S-SN, B, H) -> (B, H, S-SN) fp16
    enc_tr = np.ascontiguousarray(
        encoder_outputs[SN:].transpose(1, 2, 0)
    ).astype(np.float16)
    w16 = np.ascontiguousarray(W).astype(np.float16)
    hid16 = hidden[0].astype(np.float16)
    eye = np.eye(P, dtype=np.float32)
    selc = np.zeros((BL, BL * P), dtype=np.float16)
    for b in range(BL):
        selc[b, b * P: (b + 1) * P] = 1.0
    in_maps = []
    for c in range(NCORES):
        aux = np.concatenate(
            [hid16[c * BL: (c + 1) * BL], selc], axis=1
        )
        in_maps.append(
            {
                "enc_nat": enc_nat[c * BL: (c + 1) * BL],
                "enc_tr": enc_tr[c * BL: (c + 1) * BL].reshape(
                    BL, KC, P, S - SN
                ),
                "aux": np.ascontiguousarray(aux),
                "w": w16,
                "eye": eye,
            }
        )
    return in_maps


def kernel(hidden, encoder_outputs, W, b, **run_kwargs):
    # `b` (the nn.Linear bias) shifts every energy row by a per-batch
    # constant, which softmax cancels exactly — unused on device.
    nc = _get_nc()
    in_maps = make_in_maps(hidden, encoder_outputs, W)
    res = run_bass_kernel_spmd(
        nc, in_maps, core_ids=list(range(NCORES)), **run_kwargs
    )
    outs = [r["out"] for r in res.results]
    full = np.concatenate(outs, axis=0)  # (16, 4096)
    return full.reshape(B, 1, S).astype(np.float32)
